# revision 1
# baseline (speedup 1.0000x reference)
"""GNN message-passing (ConvGraph) Trainium2 Bass kernel, 8 NeuronCores.

Computes out = segment_sum(edge_weight * (x @ W)[edge_src], edge_dst) for a
graph with N nodes and E edges.

Strategy:
  - Shard nodes (rows of x / out) across the 8 cores; replicate W.
  - Everything except the PSUM accumulation runs in bf16 (tolerance is
    2e-2; bf16 keeps us ~3e-3).
  - Each core computes its h shard = x_m @ W on TensorE (bf16), then an
    AllGather makes the full h table [N_pad, 128] bf16 resident in every
    core's HBM.
  - Edges are partitioned by destination core, then bucketed by
    (dst block of 128 nodes, h-table chunk of 25088 rows) and padded to
    128-edge groups so the device program is static and identical across
    cores (SPMD).  Chunking keeps gather indices within int16 range.
  - Per-edge h rows are fetched with SWDGE dma_gather from HBM
    (256B bf16 rows, transpose=False -> edge-major output).  The gather
    is SWDGE-descriptor-rate bound (~7ns/row per queue); calls rotate
    across all 4 SWDGE queues and overlap.  (transpose=True gathers
    corrupt each other across queues, so the SBUF-source path is not
    usable concurrently; the HBM path is.)
  - The weighted segment-sum is one matmul per 128-edge group:
    psum[dst128, f128] += S_T[e,dst].T @ msgs[e,f].  S_T (the
    edge_weight-scaled dst one-hot, bf16) is precomputed on the HOST and
    streamed from HBM: building it on-device needs per-partition scalar
    operands which run at ~1.3-2.2us per DVE op (measured).
  - PSUM accumulates across chunks and groups per dst block (start/stop
    flags); one scalar-engine copy per block and one DMA per superblock
    write the fp32 output.
"""

import os
import sys
from contextlib import ExitStack

import numpy as np

for _p in ("/opt/trn_rl_repo",):
    if _p not in sys.path and os.path.isdir(_p):
        sys.path.insert(0, _p)

import ml_dtypes  # noqa: E402

import concourse.bass as bass  # noqa: E402
import concourse.mybir as mybir  # noqa: E402
import concourse.tile as tile  # noqa: E402
from concourse import bacc, library_config  # noqa: E402
from concourse.bass_utils import run_bass_kernel_spmd  # noqa: E402

N_CORES = 8
P = 128
D_IN = 256
D_OUT = 128
NCHUNK = 4  # h-table chunks (2 core shards per chunk; int16 idx limit)

BF16 = ml_dtypes.bfloat16


def make_cfg(n_nodes: int) -> dict:
    assert n_nodes % N_CORES == 0
    r0 = n_nodes // N_CORES
    r = ((r0 + P - 1) // P) * P
    nb = r // P
    sb = 1
    for cand in (7, 8, 6, 5, 4, 9, 10, 3, 2, 14, 1):
        if nb % cand == 0:
            sb = cand
            break
    ch = (N_CORES * r) // NCHUNK
    assert ch <= 32767, f"chunk rows {ch} exceed int16 index range"
    assert ch == 2 * r
    return dict(
        n_nodes=n_nodes, R0=r0, R=r, NB=nb, SB=sb, NSB=nb // sb, CH=ch
    )


ABLATE = os.environ.get("GNN_ABLATE", "")


def build_bass(cfg: dict, S_list: tuple):
    """Build the SPMD Bass program (same NEFF for all 8 cores)."""
    R, NB, SB, NSB, CH = cfg["R"], cfg["NB"], cfg["SB"], cfg["NSB"], cfg["CH"]
    NG = SB * NCHUNK * sum(S_list)  # total 128-edge groups per core
    S_max = max(S_list)
    TOT = NG * P  # total padded edge slots per core
    # group offset of each gather call (call = sb*NCHUNK + c)
    call_goff = []
    goff = 0
    for sb in range(NSB):
        for c in range(NCHUNK):
            call_goff.append(goff)
            goff += SB * S_list[sb]
    f32 = mybir.dt.float32
    bf16 = mybir.dt.bfloat16
    i16 = mybir.dt.int16

    nc = bacc.Bacc(
        "TRN2",
        target_bir_lowering=False,
        debug=False,
        num_devices=N_CORES,
        num_swdge_queues=4,
    )

    xT = nc.declare_dram_parameter("xT", [D_IN, R], bf16, isOutput=False)
    Wp = nc.declare_dram_parameter("W", [D_IN, D_OUT], bf16, isOutput=False)
    idxp = nc.declare_dram_parameter("idx", [P, TOT // 16], i16, isOutput=False)
    onehp = nc.declare_dram_parameter("oneh", [P, NG * P], bf16, isOutput=False)
    outp = nc.declare_dram_parameter("out", [R, D_OUT], f32, isOutput=True)

    h_shard = nc.dram_tensor("h_shard", [R, D_OUT], bf16)
    h_full = nc.dram_tensor(
        "h_full", [N_CORES * R, D_OUT], bf16, addr_space="Shared"
    )

    DK = D_IN // P  # k-chunks for the projection matmul

    with tile.TileContext(nc) as tc, ExitStack() as ctx:
        const = ctx.enter_context(tc.tile_pool(name="const", bufs=1))
        xpool = ctx.enter_context(tc.tile_pool(name="xp", bufs=2))
        hstage = ctx.enter_context(tc.tile_pool(name="hst", bufs=2))
        psum = ctx.enter_context(tc.tile_pool(name="ps", bufs=8, space="PSUM"))
        gpool = ctx.enter_context(tc.tile_pool(name="gat", bufs=5))
        ipool = ctx.enter_context(tc.tile_pool(name="idxp", bufs=8))
        opool = ctx.enter_context(tc.tile_pool(name="onehs", bufs=5))
        ostage = ctx.enter_context(tc.tile_pool(name="ost", bufs=2))

        nc.gpsimd.load_library(library_config.mlp)

        w_t = const.tile([P, DK, P], bf16)
        for k in range(DK):
            nc.sync.dma_start(out=w_t[:, k, :], in_=Wp[k * P : (k + 1) * P, :])

        # Phase A: h_shard = x_m @ W (bf16)
        TS = 4  # row-tiles per strip
        nstrip = (NB + TS - 1) // TS
        for s_ in range(nstrip):
            t0 = s_ * TS
            t1 = min(NB, t0 + TS)
            nt = t1 - t0
            xk = []
            for k in range(DK):
                xkt = xpool.tile([P, TS * P], bf16, tag=f"x{k}")
                nc.sync.dma_start(
                    out=xkt[:, : nt * P],
                    in_=xT[k * P : (k + 1) * P, t0 * P : t1 * P],
                )
                xk.append(xkt)
            hst = hstage.tile([P, TS, P], bf16, tag="hst")
            for t in range(nt):
                ps = psum.tile([P, P], f32, tag="ps")
                for k in range(DK):
                    nc.tensor.matmul(
                        ps[:],
                        xk[k][:, t * P : (t + 1) * P],
                        w_t[:, k, :],
                        start=(k == 0),
                        stop=(k == DK - 1),
                    )
                nc.scalar.copy(out=hst[:, t, :], in_=ps[:])
            nc.sync.dma_start(
                out=h_shard[t0 * P : t1 * P, :].rearrange(
                    "(t p) f -> p t f", p=P
                ),
                in_=hst[:, :nt, :],
            )

        # Phase B: AllGather h across the 8 cores (bf16).  (A pipelined
        # 4x row-quarter split was tried and measured NET-WORSE: the
        # collectives overlap the first gathers and steal HBM bandwidth
        # from the IOPS-bound random reads.)
        nc.gpsimd.collective_compute(
            "AllGather",
            mybir.AluOpType.bypass,
            ins=[h_shard[:]],
            outs=[h_full[:]],
            replica_groups=[list(range(N_CORES))],
        )

        # Phase C: gather + weighted segment-sum
        for sb in range(NSB if "nophasec" not in ABLATE else 0):
            S = S_list[sb]
            NGC = SB * S
            NI = NGC * P
            ps_blocks = [
                psum.tile([P, P], f32, tag="ps", name=f"psb_{sb}_{b}")
                for b in range(SB)
            ]
            for c in range(NCHUNK):
                call = sb * NCHUNK + c
                goff = call_goff[call]
                it = ipool.tile([P, SB * S_max * 8], i16, tag="it")
                nc.sync.dma_start(
                    out=it[:, : NI // 16],
                    in_=idxp[:, goff * 8 : goff * 8 + NI // 16],
                )
                gt = gpool.tile([P, SB * S_max, P], bf16, tag="gt")
                if "nogather" in ABLATE:
                    nc.vector.memset(gt[:], 0.0)
                else:
                    nc.gpsimd.dma_gather(
                        gt[:, :NGC, :],
                        h_full[c * CH : (c + 1) * CH, :],
                        it[:, : NI // 16],
                        NI,
                        NI,
                        P,
                        transpose=False,
                        single_packet="sp1" in ABLATE,
                        queue_num=(
                            0 if "q0" in ABLATE else call % 4
                        ),
                    )
                oh = opool.tile([P, SB * S_max, P], bf16, tag="oh")
                nc.sync.dma_start(
                    out=oh[:, :NGC, :],
                    in_=onehp[:, goff * P : goff * P + NGC * P],
                )
                for b in range(SB):
                    for j in range(S):
                        g = b * S + j
                        nc.tensor.matmul(
                            ps_blocks[b][:],
                            oh[:, g, :],
                            gt[:, g, :],
                            start=(c == 0 and j == 0),
                            stop=(c == NCHUNK - 1 and j == S - 1),
                        )
            ot = ostage.tile([P, SB, P], f32, tag="ot")
            for b in range(SB):
                nc.scalar.copy(out=ot[:, b, :], in_=ps_blocks[b][:])
            nc.sync.dma_start(
                out=outp[sb * SB * P : (sb + 1) * SB * P, :].rearrange(
                    "(b p) f -> p b f", p=P
                ),
                in_=ot[:],
            )
        if "nophasec" in ABLATE:
            zt = ostage.tile([P, SB, P], f32, tag="ot")
            nc.vector.memset(zt[:], 0.0)
            for sb in range(NSB):
                nc.sync.dma_start(
                    out=outp[sb * SB * P : (sb + 1) * SB * P, :].rearrange(
                        "(b p) f -> p b f", p=P
                    ),
                    in_=zt[:],
                )

    nc.compile()
    return nc


def host_prep(x, W, edge_src, edge_dst, edge_weight, cfg):
    """Shard + stage inputs. Returns (in_maps, S)."""
    R0, R, NB, SB, NSB, CH = (
        cfg["R0"], cfg["R"], cfg["NB"], cfg["SB"], cfg["NSB"], cfg["CH"]
    )
    x = np.asarray(x, dtype=np.float32)
    W = np.asarray(W, dtype=np.float32)
    edge_src = np.asarray(edge_src, dtype=np.int64)
    edge_dst = np.asarray(edge_dst, dtype=np.int64)
    edge_weight = np.asarray(edge_weight, dtype=np.float32)

    # Source-node h_full row: core m_s = n // R0, local l = n - m_s*R0;
    # h_full row = m_s*R + l; chunk c = row // CH, chunk-local idx int16.
    m_s = edge_src // R0
    l_s = edge_src - m_s * R0
    src_chunk = (m_s // 2).astype(np.int64)
    idx16_all = ((m_s % 2) * R + l_s).astype(np.int64)

    core_of = edge_dst // R0
    per_core = []
    max_count = 1
    for m in range(N_CORES):
        sel = core_of == m
        d = edge_dst[sel] - m * R0
        w = edge_weight[sel]
        b = d // P
        dstl = (d % P).astype(np.int64)
        c = src_chunk[sel]
        lidx = idx16_all[sel].astype(np.int16)
        key = (b * NCHUNK + c).astype(np.int64)
        counts = np.bincount(key, minlength=NB * NCHUNK)
        max_count = max(max_count, int(counts.max()))
        per_core.append((b, c, dstl, lidx, w, key, counts))

    # Per-superblock S: max bucket count over cores, chunks, blocks in sb.
    all_counts = np.stack([pc[6] for pc in per_core])  # [cores, NB*NCHUNK]
    cmax = all_counts.max(axis=0).reshape(NB, NCHUNK)
    S_list = tuple(
        max(1, int((cmax[sb * SB : (sb + 1) * SB].max() + P - 1) // P))
        for sb in range(NSB)
    )
    NG = SB * NCHUNK * sum(S_list)
    TOT = NG * P
    # group offset of each gather call (call = sb*NCHUNK + c)
    call_goff = np.zeros(NSB * NCHUNK, dtype=np.int64)
    goff = 0
    for sb in range(NSB):
        for c in range(NCHUNK):
            call_goff[sb * NCHUNK + c] = goff
            goff += SB * S_list[sb]
    S_arr = np.array(S_list, dtype=np.int64)

    in_maps = []
    for m in range(N_CORES):
        b, c, dstl, lidx, w, key, counts = per_core[m]
        # Device call order: superblock-major, then chunk, then block
        # within superblock, then the S groups of 128 slots.  Sort edges
        # by (block, chunk) bucket, then by gather idx within each bucket
        # (HBM page locality).
        order = np.lexsort((lidx, key))
        key_s = key[order]
        starts = np.zeros(NB * NCHUNK + 1, dtype=np.int64)
        np.cumsum(counts, out=starts[1:])
        rank = np.arange(len(key_s)) - starts[key_s]
        bb = b[order]
        cc = c[order]
        sb_of = bb // SB
        slot_base = (
            call_goff[sb_of * NCHUNK + cc] + (bb % SB) * S_arr[sb_of]
        ) * P
        slot = slot_base + rank

        idx_stream = np.zeros(TOT, dtype=np.int16)
        idx_stream[slot] = lidx[order]
        idx_wrapped = np.ascontiguousarray(
            np.tile(idx_stream.reshape(-1, 16).T, (8, 1))
        )

        # Host-precomputed weighted one-hot: for edge at slot s
        # (group g = s//128, lane e = s%128), S_T[e, g*128 + dstl] = w.
        oneh = np.zeros((P, NG * P), dtype=BF16)
        oneh[slot % P, (slot // P) * P + dstl[order]] = w[order].astype(BF16)

        x_m = np.zeros((R, D_IN), dtype=BF16)
        x_m[:R0] = x[m * R0 : (m + 1) * R0].astype(BF16)
        xT_m = np.ascontiguousarray(x_m.T)

        in_maps.append(
            {
                "xT": xT_m,
                "W": W.astype(BF16),
                "idx": idx_wrapped,
                "oneh": oneh,
            }
        )
    return in_maps, S_list


_BUILD_CACHE: dict = {}


def run(x, W, edge_src, edge_dst, edge_weight, trace=False, trace_kwargs=None):
    n_nodes = x.shape[0]
    cfg = make_cfg(n_nodes)
    in_maps, S_list = host_prep(x, W, edge_src, edge_dst, edge_weight, cfg)
    key = (n_nodes, S_list)
    if key not in _BUILD_CACHE:
        _BUILD_CACHE[key] = build_bass(cfg, S_list)
    nc = _BUILD_CACHE[key]
    res = run_bass_kernel_spmd(
        nc,
        in_maps,
        core_ids=list(range(N_CORES)),
        trace=trace,
        **(trace_kwargs or {}),
    )
    R0, R = cfg["R0"], cfg["R"]
    out = np.concatenate(
        [np.asarray(res.results[m]["out"])[:R0] for m in range(N_CORES)], axis=0
    )
    return out, res


def kernel(**inputs) -> np.ndarray:
    out, _ = run(
        inputs["x"],
        inputs["W"],
        inputs["edge_src"],
        inputs["edge_dst"],
        inputs["edge_weight"],
        trace=False,
    )
    return out

